# revision 1
# baseline (speedup 1.0000x reference)
import sys
sys.path.insert(0, '/opt/trn_rl_repo')
import numpy as np
import ml_dtypes
import concourse.bass as bass
import concourse.bacc as bacc
import concourse.mybir as mybir
import concourse.tile as tile
from concourse.bass_utils import run_bass_kernel_spmd

N, E0, H = 16384, 262144, 256
P = 128
BFNP = ml_dtypes.bfloat16
BF = mybir.dt.bfloat16
F32 = mybir.dt.float32
I32 = mybir.dt.int32
S3 = 1.0 / np.sqrt(3.0)
SH = 1.0 / 16.0
S2 = 1.0 / np.sqrt(2.0)
INV06 = 1.0 / 0.6


def _fold(inp):
    f = {}
    ln_g = inp["ln_g"].astype(np.float64)
    ln_b = inp["ln_b"].astype(np.float64)
    Wx1 = inp["W_x1"].astype(np.float64)
    f["Wx1"] = (ln_g[:, None] * Wx1).astype(np.float32).astype(BFNP)
    f["bx1"] = (inp["b_x1"] + (ln_b @ Wx1).astype(np.float32)).reshape(H, 1).astype(np.float32)
    Wx2 = inp["W_x2"].astype(np.float64) * INV06
    Wx2[:, H:2 * H] *= S3 * SH
    Wx2[:, 2 * H:] *= SH
    f["Wx2"] = Wx2.astype(np.float32).astype(BFNP)
    f["Wrbf"] = np.vstack([inp["W_rbf"], inp["b_rbf"][None, :]]).astype(BFNP)
    Wvp = inp["W_vp"].astype(np.float64).copy()
    Wvp[:, :H] *= SH
    f["Wvp"] = Wvp.astype(np.float32).astype(BFNP)
    f["Wxv1"] = inp["W_xv1"].astype(BFNP)
    f["bxv1"] = inp["b_xv1"].reshape(H, 1).astype(np.float32)
    Wxv2 = inp["W_xv2"].astype(np.float64) * INV06
    Wxv2[:, :2 * H] *= S2
    Wxv2[:, 2 * H:] *= 16.0
    f["Wxv2"] = Wxv2.astype(np.float32).astype(BFNP)
    f["o1Wv1"] = inp["o1_Wv1"].astype(BFNP)
    f["o1Wv2"] = inp["o1_Wv2"].astype(BFNP)
    f["o1Wu1"] = inp["o1_Wu1"].astype(BFNP)
    f["o1bu1"] = inp["o1_bu1"].reshape(H, 1).astype(np.float32)
    f["o1Wu2"] = (inp["o1_Wu2"].astype(np.float64) * INV06).astype(np.float32).astype(BFNP)
    f["o2Wv1a"] = np.hstack([inp["o2_Wv1"], inp["o2_Wv2"]]).astype(BFNP)
    o2Wu1 = inp["o2_Wu1"].astype(np.float64).copy()
    o2Wu1[:P, :] *= INV06
    f["o2Wu1"] = o2Wu1.astype(np.float32).astype(BFNP)
    f["o2bu1"] = inp["o2_bu1"].reshape(P, 1).astype(np.float32)
    f["o2Wu2c"] = (inp["o2_Wu2"][:, 1:2].astype(np.float64) * INV06).astype(np.float32).astype(BFNP)
    for nm in ("b_x2", "o1_bu2", "o2_bu2"):
        assert not np.any(inp[nm]), f"nonzero {nm} unsupported by folding"
    return f


def _pack(edge_index, edge_rbf, edge_vector):
    E = edge_index.shape[1]
    src = edge_index[0].astype(np.int64)
    dst = edge_index[1].astype(np.int64)
    gw = dst >> 7
    order = np.argsort(gw, kind="stable")
    gs = gw[order]
    srcs = src[order]
    dsts = dst[order]
    rbfs = edge_rbf[order]
    evs = edge_vector[order]
    cnt = np.bincount(gs, minlength=P)
    T = int(np.ceil(cnt.max() / P))
    NT = 16 * T
    startw = np.concatenate([[0], np.cumsum(cnt)[:-1]])
    r = np.arange(E) - startw[gs]
    core = (gs >> 4).astype(np.int64)
    wl = gs & 15
    tw = r >> 7
    p = r & 127
    t = wl * T + tw
    eidx = np.zeros((8, P, NT), np.int32)
    edst = np.full((8, P, NT), -1.0, np.float32)
    evp = np.zeros((8, P, 3 * NT), np.float32)
    rbfT = np.zeros((8, NT * 65, P), np.float32)
    eidx[core, p, t] = srcs.astype(np.int32)
    edst[core, p, t] = (dsts & 127).astype(np.float32)
    for c in range(3):
        evp[core, p, 3 * t + c] = evs[:, c]
    rbfT[core[:, None], (t * 65)[:, None] + np.arange(64)[None, :], p[:, None]] = rbfs
    rbfT[core, t * 65 + 64, p] = 1.0
    return T, NT, eidx, edst, evp, rbfT.astype(BFNP)


def _build(NT, T, reps=1):
    A = mybir.AluOpType
    FN = mybir.ActivationFunctionType
    nc = bacc.Bacc("TRN2", target_bir_lowering=False, debug=True)
    dp = nc.declare_dram_parameter
    x_d = dp("x", [N, H], F32, isOutput=False)
    vec_d = dp("vecf", [N, 3 * H], F32, isOutput=False)
    xo_d = dp("xown", [2048, H], F32, isOutput=False)
    vo_d = dp("vecown", [2048, 3 * H], F32, isOutput=False)
    ei_d = dp("eidx", [P, NT], I32, isOutput=False)
    ed_d = dp("edst", [P, NT], F32, isOutput=False)
    ev_d = dp("evp", [P, 3 * NT], F32, isOutput=False)
    rb_d = dp("rbfT", [NT * 65, P], BF, isOutput=False)
    Wx1_d = dp("Wx1", [H, H], BF, isOutput=False)
    bx1_d = dp("bx1", [H, 1], F32, isOutput=False)
    Wx2_d = dp("Wx2", [H, 3 * H], BF, isOutput=False)
    Wrbf_d = dp("Wrbf", [65, 3 * H], BF, isOutput=False)
    Wvp_d = dp("Wvp", [H, 2 * H], BF, isOutput=False)
    Wxv1_d = dp("Wxv1", [2 * H, H], BF, isOutput=False)
    bxv1_d = dp("bxv1", [H, 1], F32, isOutput=False)
    Wxv2_d = dp("Wxv2", [H, 3 * H], BF, isOutput=False)
    o1Wv1_d = dp("o1Wv1", [H, H], BF, isOutput=False)
    o1Wv2_d = dp("o1Wv2", [H, P], BF, isOutput=False)
    o1Wu1_d = dp("o1Wu1", [2 * H, H], BF, isOutput=False)
    o1bu1_d = dp("o1bu1", [H, 1], F32, isOutput=False)
    o1Wu2_d = dp("o1Wu2", [H, H], BF, isOutput=False)
    o2Wv1a_d = dp("o2Wv1a", [P, 129], BF, isOutput=False)
    o2Wu1_d = dp("o2Wu1", [2 * P, P], BF, isOutput=False)
    o2bu1_d = dp("o2bu1", [P, 1], F32, isOutput=False)
    o2Wu2c_d = dp("o2Wu2c", [P, 1], BF, isOutput=False)
    iota_d = dp("iota", [P, P], F32, isOutput=False)
    id_d = dp("ident", [P, P], BF, isOutput=False)
    out_d = dp("out3", [2048, 3], F32, isOutput=True)

    with tile.TileContext(nc) as tc:
        with tc.tile_pool(name="persist", bufs=1) as PR, \
             tc.tile_pool(name="dpool", bufs=1, space="DRAM") as DP:
            def ld(nm, src_ap, shape, dt):
                tl = PR.tile(shape, dt, tag=nm, name=nm)
                nc.sync.dma_start(out=tl[:], in_=src_ap)
                return tl

            wx1 = [ld(f"wx1_{k}", Wx1_d[k * P:(k + 1) * P, :], [P, H], BF) for k in range(2)]
            bx1 = [ld(f"bx1_{k}", bx1_d[k * P:(k + 1) * P, :], [P, 1], F32) for k in range(2)]
            wx2 = [ld(f"wx2_{k}", Wx2_d[k * P:(k + 1) * P, :], [P, 3 * H], BF) for k in range(2)]
            wrbf = ld("wrbf", Wrbf_d[:, :], [65, 3 * H], BF)
            wvp = [ld(f"wvp_{k}", Wvp_d[k * P:(k + 1) * P, :], [P, 2 * H], BF) for k in range(2)]
            wxv1 = [ld(f"wxv1_{k}", Wxv1_d[k * P:(k + 1) * P, :], [P, H], BF) for k in range(4)]
            bxv1 = [ld(f"bxv1_{k}", bxv1_d[k * P:(k + 1) * P, :], [P, 1], F32) for k in range(2)]
            wxv2 = [ld(f"wxv2_{k}", Wxv2_d[k * P:(k + 1) * P, :], [P, 3 * H], BF) for k in range(2)]
            o1wv1 = [ld(f"o1wv1_{k}", o1Wv1_d[k * P:(k + 1) * P, :], [P, H], BF) for k in range(2)]
            o1wv2 = [ld(f"o1wv2_{k}", o1Wv2_d[k * P:(k + 1) * P, :], [P, P], BF) for k in range(2)]
            o1wu1 = [ld(f"o1wu1_{k}", o1Wu1_d[k * P:(k + 1) * P, :], [P, H], BF) for k in range(4)]
            o1bu1 = [ld(f"o1bu1_{k}", o1bu1_d[k * P:(k + 1) * P, :], [P, 1], F32) for k in range(2)]
            o1wu2 = [ld(f"o1wu2_{k}", o1Wu2_d[k * P:(k + 1) * P, :], [P, H], BF) for k in range(2)]
            o2wv1a = ld("o2wv1a", o2Wv1a_d[:, :], [P, 129], BF)
            o2wu1 = [ld(f"o2wu1_{k}", o2Wu1_d[k * P:(k + 1) * P, :], [P, P], BF) for k in range(2)]
            o2bu1 = ld("o2bu1", o2bu1_d[:, :], [P, 1], F32)
            o2wu2c = ld("o2wu2c", o2Wu2c_d[:, :], [P, 1], BF)
            iota = ld("iota", iota_d[:, :], [P, P], F32)
            idb = ld("idb", id_d[:, :], [P, P], BF)
            eidx = ld("eidx", ei_d[:, :], [P, NT], I32)
            edst = ld("edst", ed_d[:, :], [P, NT], F32)
            evp = ld("evp", ev_d[:, :], [P, 3 * NT], F32)
            eps5 = PR.tile([P, 1], F32, tag="eps5", name="eps5")
            nc.vector.memset(eps5[:], 1e-5)
            eps8 = PR.tile([P, 1], F32, tag="eps8", name="eps8")
            nc.vector.memset(eps8[:], 1e-8)
            x1sb = PR.tile([P, 16 * H], F32, tag="x1sb", name="x1sb")
            v1sb = PR.tile([P, 48 * H], F32, tag="v1sb", name="v1sb")
            gsrc = DP.tile([N, 5 * H], BF, tag="gsrc", name="gsrc")

            for rep in range(reps):
              # ---------- phase 1: g-pack for all nodes (replicated) ----------
              with tc.tile_pool(name=f"p1_{rep}", bufs=2) as S1, \
                   tc.tile_pool(name=f"q1_{rep}", bufs=2, space="PSUM") as Q1:
                for i in range(N // P):
                    xt = S1.tile([P, H], F32, tag="xt", bufs=3)
                    nc.sync.dma_start(out=xt[:], in_=x_d[i * P:(i + 1) * P, :])
                    vt = S1.tile([P, 3 * H], F32, tag="vt", bufs=3)
                    nc.sync.dma_start(out=vt[:], in_=vec_d[i * P:(i + 1) * P, :])
                    st6 = S1.tile([P, 6], F32, tag="st6", bufs=3)
                    nc.vector.bn_stats(out=st6[:], in_=xt[:])
                    mv = S1.tile([P, 2], F32, tag="mv", bufs=3)
                    nc.vector.bn_aggr(out=mv[:], in_=st6[:])
                    sd = S1.tile([P, 1], F32, tag="sd", bufs=3)
                    nc.scalar.activation(out=sd[:], in_=mv[:, 1:2], func=FN.Sqrt, bias=eps5[:, 0:1])
                    ri = S1.tile([P, 1], F32, tag="ri", bufs=3)
                    nc.vector.reciprocal(out=ri[:], in_=sd[:])
                    xh = S1.tile([P, H], BF, tag="xh", bufs=3)
                    nc.vector.tensor_scalar(out=xh[:], in0=xt[:], scalar1=mv[:, 0:1],
                                            scalar2=ri[:, 0:1], op0=A.subtract, op1=A.mult)
                    xhT = S1.tile([P, H], BF, tag="xhT", bufs=3)
                    for k in range(2):
                        pt = Q1.tile([P, P], BF, tag="p1tp")
                        nc.tensor.transpose(out=pt[:], in_=xh[:, k * P:(k + 1) * P], identity=idb[:])
                        nc.scalar.activation(out=xhT[:, k * P:(k + 1) * P], in_=pt[:], func=FN.Copy)
                    hT = Q1.tile([P, H], F32, tag="p1h")
                    for mo in range(2):
                        for k in range(2):
                            nc.tensor.matmul(out=hT[:, mo * P:(mo + 1) * P],
                                             lhsT=wx1[k][:, mo * P:(mo + 1) * P],
                                             rhs=xhT[:, k * P:(k + 1) * P],
                                             start=(k == 0), stop=(k == 1))
                    silT = S1.tile([P, H], BF, tag="silT", bufs=3)
                    for mo in range(2):
                        nc.scalar.activation(out=silT[:, mo * P:(mo + 1) * P],
                                             in_=hT[:, mo * P:(mo + 1) * P],
                                             func=FN.Silu, bias=bx1[mo][:, 0:1])
                    pA = Q1.tile([P, 2 * H], F32, tag="p1a")
                    pB = Q1.tile([P, H], F32, tag="p1b")
                    for mo in range(2):
                        nc.tensor.matmul(out=pA[:], lhsT=silT[:, mo * P:(mo + 1) * P],
                                         rhs=wx2[mo][:, 0:2 * H], start=(mo == 0), stop=(mo == 1))
                        nc.tensor.matmul(out=pB[:], lhsT=silT[:, mo * P:(mo + 1) * P],
                                         rhs=wx2[mo][:, 2 * H:3 * H], start=(mo == 0), stop=(mo == 1))
                    gp = S1.tile([P, 5 * H], BF, tag="gp", bufs=3)
                    nc.scalar.activation(out=gp[:, 0:H], in_=pA[:, 0:H], func=FN.Copy)
                    nc.scalar.activation(out=gp[:, H:2 * H], in_=pB[:, 0:H], func=FN.Copy)
                    for c in range(3):
                        nc.vector.tensor_tensor(out=gp[:, (2 + c) * H:(3 + c) * H],
                                                in0=vt[:, c * H:(c + 1) * H],
                                                in1=pA[:, H:2 * H], op=A.mult)
                    nc.sync.dma_start(out=gsrc[i * P:(i + 1) * P, :], in_=gp[:])

            # ---------- phase 2: gather + edge messages + windowed scatter ----------
              with tc.tile_pool(name=f"p2_{rep}", bufs=2) as S2, \
                   tc.tile_pool(name=f"q2_{rep}", bufs=2, space="PSUM") as Q2:
                for w in range(16):
                    winA = Q2.tile([P, 512], F32, tag="winA", bufs=1)
                    winB = Q2.tile([P, 512], F32, tag="winB", bufs=1)
                    wC1 = Q2.tile([P, 512], F32, tag="wC1", bufs=1)
                    wC2 = Q2.tile([P, 256], F32, tag="wC2", bufs=1)
                    for tw in range(T):
                        t = w * T + tw
                        gt = S2.tile([P, 5 * H], BF, tag="gt", bufs=4)
                        nc.gpsimd.indirect_dma_start(
                            out=gt[:], out_offset=None, in_=gsrc[:],
                            in_offset=bass.IndirectOffsetOnAxis(ap=eidx[:, t:t + 1], axis=0))
                        rbt = S2.tile([65, P], BF, tag="rbt", bufs=4)
                        nc.sync.dma_start(out=rbt[:], in_=rb_d[t * 65:(t + 1) * 65, :])
                        rA = Q2.tile([P, 2 * H], F32, tag="rA", bufs=2)
                        rB = Q2.tile([P, H], F32, tag="rB", bufs=2)
                        nc.tensor.matmul(out=rA[:], lhsT=rbt[:], rhs=wrbf[:, 0:2 * H],
                                         start=True, stop=True)
                        nc.tensor.matmul(out=rB[:], lhsT=rbt[:], rhs=wrbf[:, 2 * H:3 * H],
                                         start=True, stop=True)
                        St = S2.tile([P, P], BF, tag="St", bufs=3)
                        nc.vector.tensor_scalar(out=St[:], in0=iota[:], scalar1=edst[:, t:t + 1],
                                                scalar2=None, op0=A.is_equal)
                        mall = S2.tile([P, 4 * H], BF, tag="mall", bufs=3)
                        nc.vector.tensor_tensor(out=mall[:, 0:H], in0=gt[:, 0:H],
                                                in1=rA[:, 0:H], op=A.mult)
                        for c in range(3):
                            nc.vector.tensor_tensor(out=mall[:, (1 + c) * H:(2 + c) * H],
                                                    in0=gt[:, (2 + c) * H:(3 + c) * H],
                                                    in1=rA[:, H:2 * H], op=A.mult)
                        m3 = S2.tile([P, H], BF, tag="m3", bufs=3)
                        nc.vector.tensor_tensor(out=m3[:], in0=gt[:, H:2 * H],
                                                in1=rB[:, 0:H], op=A.mult)
                        m3e = S2.tile([P, 3 * H], BF, tag="m3e", bufs=3)
                        for c in range(3):
                            nc.vector.tensor_scalar_mul(m3e[:, c * H:(c + 1) * H], m3[:],
                                                        evp[:, 3 * t + c:3 * t + c + 1])
                        st0, sp0 = (tw == 0), (tw == T - 1)
                        nc.tensor.matmul(out=winA[:], lhsT=St[:], rhs=mall[:, 0:512],
                                         start=st0, stop=sp0)
                        nc.tensor.matmul(out=winB[:], lhsT=St[:], rhs=mall[:, 512:1024],
                                         start=st0, stop=sp0)
                        nc.tensor.matmul(out=wC1[:], lhsT=St[:], rhs=m3e[:, 0:512],
                                         start=st0, stop=sp0)
                        nc.tensor.matmul(out=wC2[:], lhsT=St[:], rhs=m3e[:, 512:768],
                                         start=st0, stop=sp0)
                    xo = S2.tile([P, H], F32, tag="xo")
                    nc.sync.dma_start(out=xo[:], in_=xo_d[w * P:(w + 1) * P, :])
                    vo = S2.tile([P, 3 * H], F32, tag="vo")
                    nc.sync.dma_start(out=vo[:], in_=vo_d[w * P:(w + 1) * P, :])
                    nc.vector.tensor_add(out=x1sb[:, w * H:(w + 1) * H], in0=xo[:],
                                         in1=winA[:, 0:H])
                    dv = [winA[:, H:2 * H], winB[:, 0:H], winB[:, H:2 * H]]
                    mc = [wC1[:, 0:H], wC1[:, H:2 * H], wC2[:, 0:H]]
                    for c in range(3):
                        td = S2.tile([P, H], F32, tag="td")
                        nc.vector.tensor_add(out=td[:], in0=vo[:, c * H:(c + 1) * H], in1=dv[c])
                        nc.vector.tensor_add(out=v1sb[:, w * 3 * H + c * H:w * 3 * H + (c + 1) * H],
                                             in0=td[:], in1=mc[c])

            # ---------- phase 3: update + output blocks on own nodes ----------
              with tc.tile_pool(name=f"p3_{rep}", bufs=2) as S3, \
                   tc.tile_pool(name=f"q3_{rep}", bufs=2, space="PSUM") as Q3:
                for j in range(16):
                    x1 = x1sb[:, j * H:(j + 1) * H]
                    v1 = [v1sb[:, j * 3 * H + c * H:j * 3 * H + (c + 1) * H] for c in range(3)]

                    def tr2(src_bf, dst_bf, nk):
                        for k in range(nk):
                            pt = Q3.tile([P, P], BF, tag="ppt", bufs=3)
                            nc.tensor.transpose(out=pt[:], in_=src_bf[:, k * P:(k + 1) * P],
                                                identity=idb[:])
                            nc.scalar.activation(out=dst_bf[:, k * P:(k + 1) * P], in_=pt[:],
                                                 func=FN.Copy)

                    # vp = vec1 @ Wvp ; evac vp1/vp2
                    vp1s, vp2s = [], []
                    for c in range(3):
                        vbfc = S3.tile([P, H], BF, tag=f"v1bf{c}")
                        nc.scalar.activation(out=vbfc[:], in_=v1[c], func=FN.Copy)
                        vTc = S3.tile([P, H], BF, tag=f"v1T{c}")
                        tr2(vbfc, vTc, 2)
                        vpp = Q3.tile([P, 2 * H], F32, tag="pp", bufs=5)
                        for k in range(2):
                            nc.tensor.matmul(out=vpp[:], lhsT=vTc[:, k * P:(k + 1) * P],
                                             rhs=wvp[k][:, :], start=(k == 0), stop=(k == 1))
                        v1p = S3.tile([P, H], F32, tag=f"vp1{c}")
                        nc.scalar.activation(out=v1p[:], in_=vpp[:, 0:H], func=FN.Copy)
                        v2p = S3.tile([P, H], F32, tag=f"vp2{c}")
                        nc.scalar.activation(out=v2p[:], in_=vpp[:, H:2 * H], func=FN.Copy)
                        vp1s.append(v1p)
                        vp2s.append(v2p)
                    # vec_dot and vnorm
                    vd = S3.tile([P, H], F32, tag="vd")
                    nc.vector.tensor_tensor(out=vd[:], in0=vp1s[0][:], in1=vp2s[0][:], op=A.mult)
                    for c in (1, 2):
                        vdt = S3.tile([P, H], F32, tag="vdt")
                        nc.vector.tensor_tensor(out=vdt[:], in0=vp1s[c][:], in1=vp2s[c][:], op=A.mult)
                        nc.vector.tensor_add(out=vd[:], in0=vd[:], in1=vdt[:])
                    vns = S3.tile([P, H], F32, tag="vns")
                    sq0 = S3.tile([P, H], F32, tag="sq0")
                    nc.scalar.activation(out=sq0[:], in_=vp2s[0][:], func=FN.Square)
                    sq1 = S3.tile([P, H], F32, tag="sq1")
                    nc.scalar.activation(out=sq1[:], in_=vp2s[1][:], func=FN.Square)
                    nc.vector.tensor_add(out=vns[:], in0=sq0[:], in1=sq1[:])
                    sq2 = S3.tile([P, H], F32, tag="sq2")
                    nc.scalar.activation(out=sq2[:], in_=vp2s[2][:], func=FN.Square)
                    nc.vector.tensor_add(out=vns[:], in0=vns[:], in1=sq2[:])
                    vn = S3.tile([P, H], BF, tag="vn")
                    nc.scalar.activation(out=vn[:], in_=vns[:], func=FN.Sqrt, bias=eps8[:, 0:1])
                    # MLP-U
                    xb = S3.tile([P, H], BF, tag="xb")
                    nc.scalar.activation(out=xb[:], in_=x1, func=FN.Copy)
                    inT = S3.tile([P, 2 * H], BF, tag="inT")
                    tr2(xb, inT, 2)
                    inTv = inT[:, H:2 * H]
                    for k in range(2):
                        pt = Q3.tile([P, P], BF, tag="ppt", bufs=3)
                        nc.tensor.transpose(out=pt[:], in_=vn[:, k * P:(k + 1) * P], identity=idb[:])
                        nc.scalar.activation(out=inTv[:, k * P:(k + 1) * P], in_=pt[:], func=FN.Copy)
                    hTu = Q3.tile([P, H], F32, tag="pp", bufs=5)
                    for mo in range(2):
                        for k in range(4):
                            nc.tensor.matmul(out=hTu[:, mo * P:(mo + 1) * P],
                                             lhsT=wxv1[k][:, mo * P:(mo + 1) * P],
                                             rhs=inT[:, k * P:(k + 1) * P],
                                             start=(k == 0), stop=(k == 3))
                    silu = S3.tile([P, H], BF, tag="silu")
                    for mo in range(2):
                        nc.scalar.activation(out=silu[:, mo * P:(mo + 1) * P],
                                             in_=hTu[:, mo * P:(mo + 1) * P],
                                             func=FN.Silu, bias=bxv1[mo][:, 0:1])
                    pA2 = Q3.tile([P, 2 * H], F32, tag="pp", bufs=5)
                    pB2 = Q3.tile([P, H], F32, tag="pp", bufs=5)
                    for mo in range(2):
                        nc.tensor.matmul(out=pA2[:], lhsT=silu[:, mo * P:(mo + 1) * P],
                                         rhs=wxv2[mo][:, 0:2 * H], start=(mo == 0), stop=(mo == 1))
                        nc.tensor.matmul(out=pB2[:], lhsT=silu[:, mo * P:(mo + 1) * P],
                                         rhs=wxv2[mo][:, 2 * H:3 * H], start=(mo == 0), stop=(mo == 1))
                    xv3 = S3.tile([P, H], F32, tag="xv3")
                    nc.scalar.activation(out=xv3[:], in_=pB2[:, 0:H], func=FN.Copy)
                    # x2 = x1 + xv1' + xv2' * vd   (in sbuf tile x2)
                    tg = S3.tile([P, H], F32, tag="tg")
                    nc.vector.tensor_tensor(out=tg[:], in0=vd[:], in1=pA2[:, H:2 * H], op=A.mult)
                    x2 = S3.tile([P, H], F32, tag="x2")
                    nc.vector.tensor_tensor(out=x2[:], in0=x1, in1=pA2[:, 0:H], op=A.add)
                    nc.vector.tensor_add(out=x2[:], in0=x2[:], in1=tg[:])
                    # vec2_c = v1_c + xv3 * vp1_c  (overwrite v1sb)
                    for c in range(3):
                        tm = S3.tile([P, H], F32, tag=f"tm{c}")
                        nc.vector.tensor_tensor(out=tm[:], in0=xv3[:], in1=vp1s[c][:], op=A.mult)
                        nc.vector.tensor_add(out=v1[c], in0=v1[c], in1=tm[:])
                    # ---- gated block 1 ----
                    v2T = []
                    for c in range(3):
                        v2bf = S3.tile([P, H], BF, tag=f"v2bf{c}")
                        nc.scalar.activation(out=v2bf[:], in_=v1[c], func=FN.Copy)
                        vT = S3.tile([P, H], BF, tag=f"v2T{c}")
                        tr2(v2bf, vT, 2)
                        v2T.append(vT)
                    vns1 = S3.tile([P, H], F32, tag="vns1")
                    sqw = []
                    for c in range(3):
                        pw1 = Q3.tile([P, H], F32, tag="pp", bufs=5)
                        for k in range(2):
                            nc.tensor.matmul(out=pw1[:], lhsT=v2T[c][:, k * P:(k + 1) * P],
                                             rhs=o1wv1[k][:, :], start=(k == 0), stop=(k == 1))
                        sw = S3.tile([P, H], F32, tag=f"sqw{c}")
                        nc.scalar.activation(out=sw[:], in_=pw1[:], func=FN.Square)
                        sqw.append(sw)
                    nc.vector.tensor_add(out=vns1[:], in0=sqw[0][:], in1=sqw[1][:])
                    nc.vector.tensor_add(out=vns1[:], in0=vns1[:], in1=sqw[2][:])
                    vn1 = S3.tile([P, H], BF, tag="vn1")
                    nc.scalar.activation(out=vn1[:], in_=vns1[:], func=FN.Sqrt)
                    v2b = []
                    for c in range(3):
                        pv = Q3.tile([P, P], F32, tag="pp", bufs=5)
                        for k in range(2):
                            nc.tensor.matmul(out=pv[:], lhsT=v2T[c][:, k * P:(k + 1) * P],
                                             rhs=o1wv2[k][:, :], start=(k == 0), stop=(k == 1))
                        vb = S3.tile([P, P], F32, tag=f"v2b{c}")
                        nc.scalar.activation(out=vb[:], in_=pv[:], func=FN.Copy)
                        v2b.append(vb)
                    x2b = S3.tile([P, H], BF, tag="x2b")
                    nc.scalar.activation(out=x2b[:], in_=x2[:], func=FN.Copy)
                    inT1 = S3.tile([P, 2 * H], BF, tag="inT1")
                    tr2(x2b, inT1, 2)
                    inT1v = inT1[:, H:2 * H]
                    for k in range(2):
                        pt = Q3.tile([P, P], BF, tag="ppt", bufs=3)
                        nc.tensor.transpose(out=pt[:], in_=vn1[:, k * P:(k + 1) * P], identity=idb[:])
                        nc.scalar.activation(out=inT1v[:, k * P:(k + 1) * P], in_=pt[:], func=FN.Copy)
                    hT1 = Q3.tile([P, H], F32, tag="pp", bufs=5)
                    for mo in range(2):
                        for k in range(4):
                            nc.tensor.matmul(out=hT1[:, mo * P:(mo + 1) * P],
                                             lhsT=o1wu1[k][:, mo * P:(mo + 1) * P],
                                             rhs=inT1[:, k * P:(k + 1) * P],
                                             start=(k == 0), stop=(k == 3))
                    sil1 = S3.tile([P, H], BF, tag="sil1")
                    for mo in range(2):
                        nc.scalar.activation(out=sil1[:, mo * P:(mo + 1) * P],
                                             in_=hT1[:, mo * P:(mo + 1) * P],
                                             func=FN.Silu, bias=o1bu1[mo][:, 0:1])
                    ph1 = Q3.tile([P, H], F32, tag="pp", bufs=5)
                    for mo in range(2):
                        nc.tensor.matmul(out=ph1[:], lhsT=sil1[:, mo * P:(mo + 1) * P],
                                         rhs=o1wu2[mo][:, :], start=(mo == 0), stop=(mo == 1))
                    xn = S3.tile([P, P], BF, tag="xn")
                    nc.scalar.activation(out=xn[:], in_=ph1[:, 0:P], func=FN.Silu)
                    vnb = []
                    for c in range(3):
                        vb = S3.tile([P, P], BF, tag=f"vnb{c}")
                        nc.vector.tensor_tensor(out=vb[:], in0=v2b[c][:], in1=ph1[:, P:2 * P],
                                                op=A.mult)
                        vnb.append(vb)
                    # ---- gated block 2 ----
                    sq2b, v2f = [], []
                    for c in range(3):
                        vbT = S3.tile([P, P], BF, tag=f"vnbT{c}")
                        tr2(vnb[c], vbT, 1)
                        pw2 = Q3.tile([P, 129], F32, tag="pp", bufs=5)
                        nc.tensor.matmul(out=pw2[:], lhsT=vbT[:], rhs=o2wv1a[:, :],
                                         start=True, stop=True)
                        sb = S3.tile([P, P], F32, tag=f"sq2b{c}")
                        nc.scalar.activation(out=sb[:], in_=pw2[:, 0:P], func=FN.Square)
                        sq2b.append(sb)
                        vf = S3.tile([P, 1], F32, tag=f"v2f{c}")
                        nc.scalar.activation(out=vf[:], in_=pw2[:, P:P + 1], func=FN.Copy)
                        v2f.append(vf)
                    vns2 = S3.tile([P, P], F32, tag="vns2")
                    nc.vector.tensor_add(out=vns2[:], in0=sq2b[0][:], in1=sq2b[1][:])
                    nc.vector.tensor_add(out=vns2[:], in0=vns2[:], in1=sq2b[2][:])
                    vn2 = S3.tile([P, P], BF, tag="vn2")
                    nc.scalar.activation(out=vn2[:], in_=vns2[:], func=FN.Sqrt)
                    xnT = S3.tile([P, P], BF, tag="xnT")
                    tr2(xn, xnT, 1)
                    vn2T = S3.tile([P, P], BF, tag="vn2T")
                    tr2(vn2, vn2T, 1)
                    h2T = Q3.tile([P, P], F32, tag="pp", bufs=5)
                    nc.tensor.matmul(out=h2T[:], lhsT=o2wu1[0][:, :], rhs=xnT[:],
                                     start=True, stop=False)
                    nc.tensor.matmul(out=h2T[:], lhsT=o2wu1[1][:, :], rhs=vn2T[:],
                                     start=False, stop=True)
                    sil2 = S3.tile([P, P], BF, tag="sil2")
                    nc.scalar.activation(out=sil2[:], in_=h2T[:], func=FN.Silu, bias=o2bu1[:, 0:1])
                    phb = Q3.tile([P, 1], F32, tag="pp", bufs=5)
                    nc.tensor.matmul(out=phb[:], lhsT=sil2[:], rhs=o2wu2c[:, :],
                                     start=True, stop=True)
                    hbs = S3.tile([P, 1], F32, tag="hbs")
                    nc.scalar.activation(out=hbs[:], in_=phb[:], func=FN.Copy)
                    ot = S3.tile([P, 3], F32, tag="ot")
                    for c in range(3):
                        nc.vector.tensor_tensor(out=ot[:, c:c + 1], in0=v2f[c][:, 0:1],
                                                in1=hbs[:, 0:1], op=A.mult)
                    nc.sync.dma_start(out=out_d[j * P:(j + 1) * P, :], in_=ot[:])

    nc.compile()
    return nc


def kernel(**inputs):
    f = _fold(inputs)
    T, NT, eidx, edst, evp, rbfT = _pack(
        inputs["edge_index"], inputs["edge_rbf"], inputs["edge_vector"])
    nc = _build(NT, T)
    x = np.ascontiguousarray(inputs["x"].astype(np.float32))
    vecf = np.ascontiguousarray(inputs["vec"].astype(np.float32).reshape(N, 3 * H))
    iota = np.broadcast_to(np.arange(P, dtype=np.float32)[None, :], (P, P)).copy()
    ident = np.eye(P, dtype=np.float32).astype(BFNP)
    ins = []
    for c in range(8):
        d = {
            "x": x, "vecf": vecf,
            "xown": x[c * 2048:(c + 1) * 2048], "vecown": vecf[c * 2048:(c + 1) * 2048],
            "eidx": eidx[c], "edst": edst[c], "evp": evp[c], "rbfT": rbfT[c],
            "iota": iota, "ident": ident,
        }
        d.update(f)
        ins.append(d)
    res = run_bass_kernel_spmd(nc, ins, list(range(8)))
    out = np.concatenate([res.results[c]["out3"] for c in range(8)], axis=0)
    return out.astype(np.float32)

